# revision 1
# baseline (speedup 1.0000x reference)
"""Trainium2 Bass kernel for EfficientAttention (linear attention block).

Computation (per batch b, head h):
    qkv = x @ w_qkv.T + b_qkv
    q = softmax(q, axis=head_dim) * head_dim**-0.5
    k = softmax(k, axis=seqlen)
    kv[d,e] = sum_s k[s,d] v[s,e]          (per-head 64x64 state)
    out[s,e] = sum_d q[s,d] kv[d,e]
    y = out @ w_proj.T + b_proj

Sharding: 8 cores = (batch b = c//2, seq half = c%2); 2048 tokens per core,
all 16 heads. The only cross-core coupling is the kv state and the
k-softmax denominator Z (sums over the full 4096 seqlen) -> one small
AllReduce (pairs of cores) of [128, 1032] fp32.

Layout strategy: host pre-transposes x and weights so every matmul operand
loads naturally (d-major on partitions); activations stay token-major; the
only on-chip transpose is q_soft (needed d-major for q@kv), done on the PE.
Matmuls run as float32r (full PE rate for free dim >= 256).
"""

import os
import sys

sys.path.insert(0, "/opt/trn_rl_repo")

import numpy as np

import concourse.bacc as bacc
import concourse.tile as tile
from concourse import mybir
from concourse import bass_utils

F32 = mybir.dt.float32
F32R = mybir.dt.float32r

D = 1024          # model dim (= qkv contraction dim)
T = 2048          # tokens per core (one batch element's half sequence)
NH = 16           # heads
HD = 64           # head dim
NPAIR = 8         # head pairs (2 heads / 128 partitions)
KC = D // 128     # contraction chunks of 128
TB = T // 128     # token blocks of 128
SCALE = HD ** -0.5

N_CORES = 8


def build_program(debug=False, no_collective=False, with_bias=True):
    nc = bacc.Bacc("TRN2", target_bir_lowering=False, num_devices=N_CORES)

    xt = nc.dram_tensor("xt", [D, T], F32R, kind="ExternalInput")      # x chunk, transposed
    wq = nc.dram_tensor("wq", [D, D], F32R, kind="ExternalInput")      # w_q.T
    wk = nc.dram_tensor("wk", [D, D], F32R, kind="ExternalInput")      # w_k.T
    wv = nc.dram_tensor("wv", [D, D], F32R, kind="ExternalInput")      # w_v.T
    wp = nc.dram_tensor("wp", [D, D], F32R, kind="ExternalInput")      # w_proj.T
    bq = nc.dram_tensor("bq", [D], F32, kind="ExternalInput")
    bk = nc.dram_tensor("bk", [D], F32, kind="ExternalInput")
    bv = nc.dram_tensor("bv", [D], F32, kind="ExternalInput")
    bp = nc.dram_tensor("bp", [D], F32, kind="ExternalInput")
    cst = nc.dram_tensor("cst", [128, 132], F32R, kind="ExternalInput")  # identity | ones | pad
    y = nc.dram_tensor("y", [T, D], F32, kind="ExternalOutput")
    if debug:
        dbg_kv = nc.dram_tensor("dbg_kv", [128, 520], F32, kind="ExternalOutput")
        dbg_eq = nc.dram_tensor("dbg_eq", [128, D], F32, kind="ExternalOutput")
        dbg_at = nc.dram_tensor("dbg_at", [128, 512], F32, kind="ExternalOutput")
        dbg_ek = nc.dram_tensor("dbg_ek", [128, D], F32, kind="ExternalOutput")

    # [D, T] -> [128, kc, tokens] view for natural per-chunk loads
    xt_v = xt.rearrange("(kc p) t -> p kc t", p=128)
    wq_v = wq.rearrange("(kc p) f -> p kc f", p=128)
    wk_v = wk.rearrange("(kc p) f -> p kc f", p=128)
    wv_v = wv.rearrange("(kc p) f -> p kc f", p=128)
    wp_v = wp.rearrange("(kc p) f -> p kc f", p=128)

    def bias_bcast(b):
        # DRAM [D] broadcast-load to SBUF [128, D] (partition step 0)
        import concourse.bass as bass
        ap = b[:]
        return bass.AP(tensor=ap.tensor, offset=ap.offset, ap=[[0, 128]] + list(ap.ap))

    with tile.TileContext(nc) as tc:
        with (
            tc.tile_pool(name="const", bufs=1) as const,
            tc.tile_pool(name="wpool", bufs=2) as wpool,
            tc.tile_pool(name="xin", bufs=2) as xin,
            tc.tile_pool(name="ekv", bufs=2) as ekv,
            tc.tile_pool(name="acc", bufs=1) as accp,
            tc.tile_pool(name="qpool", bufs=2) as qpool,
            tc.tile_pool(name="qt", bufs=1) as qtpool,
            tc.tile_pool(name="kvsb", bufs=1) as kvsbp,
            tc.tile_pool(name="yout", bufs=2) as youtp,
            tc.tile_pool(name="psum", bufs=3, space="PSUM") as psum,
            tc.tile_pool(name="dram", bufs=1, space="DRAM") as dram,
        ):
            cst_sb = const.tile([128, 132], F32R, tag="cst")
            nc.sync.dma_start(cst_sb, cst[:])
            ident = cst_sb[:, 0:128]
            ones = cst_sb[:, 128:129]

            if with_bias:
                BF16 = mybir.dt.bfloat16
                bk_sb = const.tile([128, D], BF16, tag="bk")
                bv_sb = const.tile([128, D], BF16, tag="bv")
                bq_sb = const.tile([128, D], BF16, tag="bq")
                bp_sb = const.tile([128, D], BF16, tag="bp")
                nc.gpsimd.dma_start(bk_sb, bias_bcast(bk))
                nc.gpsimd.dma_start(bv_sb, bias_bcast(bv))
                nc.gpsimd.dma_start(bq_sb, bias_bcast(bq))
                nc.gpsimd.dma_start(bp_sb, bias_bcast(bp))

            wk_sb = [wpool.tile([128, D], F32R, tag=f"w{kc}", name=f"wk{kc}")
                     for kc in range(KC)]
            wv_sb = [wpool.tile([128, D], F32R, tag=f"w{kc}", name=f"wv{kc}")
                     for kc in range(KC)]
            for kc in range(KC):
                nc.sync.dma_start(wk_sb[kc], wk_v[:, kc, :])
                nc.gpsimd.dma_start(wv_sb[kc], wv_v[:, kc, :])

            # ---- Phase 1: k/v projections, exp(k), partial kv-state ----
            # kv-state accumulator in SBUF: quad q at cols [512q : 512q+512],
            # pairs 2q (cols 0:256) / 2q+1 (256:512) inside it. PSUM cross-tb
            # accumulation is NOT used: two interleaved accumulation groups in
            # one PSUM bank corrupt the first group (measured on HW), so each
            # tb does single-shot matmuls into transient PSUM + a DVE add.
            kvacc = accp.tile([128, 2048], F32, tag="kvacc")
            zps = [psum.tile([128, 512], F32, tag=f"z{h}", name=f"zps{h}", bufs=1)
                   for h in range(2)]

            for tb in range(TB):
                xtile = xin.tile([128, KC, 128], F32R, tag="x")
                nc.sync.dma_start(xtile, xt_v[:, :, tb * 128:(tb + 1) * 128])
                ek = ekv.tile([128, D], F32R, tag="ek")
                vv = ekv.tile([128, D], F32R, tag="v")
                for half in range(2):
                    sl = slice(half * 512, (half + 1) * 512)
                    # k chunk
                    ps = psum.tile([128, 512], F32, tag="mm")
                    for kc in range(KC):
                        nc.tensor.matmul(ps, (xtile[:, kc, :]), (wk_sb[kc][:, sl]),
                                         start=(kc == 0), stop=(kc == KC - 1))
                    if with_bias:
                        nc.vector.tensor_add(ps, ps, bk_sb[:, sl])
                    nc.scalar.activation(ek[:, sl], ps, mybir.ActivationFunctionType.Exp)
                    # Z partial: ones.T @ ek chunk accumulated in PSUM across
                    # tbs (single accumulation group alone in its bank = safe)
                    nc.tensor.matmul(zps[half][0:1, :], (ones), (ek[:, sl]),
                                     start=(tb == 0), stop=(tb == TB - 1))
                    # v chunk
                    ps = psum.tile([128, 512], F32, tag="mm")
                    for kc in range(KC):
                        nc.tensor.matmul(ps, (xtile[:, kc, :]), (wv_sb[kc][:, sl]),
                                         start=(kc == 0), stop=(kc == KC - 1))
                    if with_bias:
                        nc.vector.tensor_add(vv[:, sl], ps, bv_sb[:, sl])
                    else:
                        nc.scalar.copy(vv[:, sl], ps)
                if debug and tb == 0:
                    nc.sync.dma_start(dbg_ek[:], ek[:].bitcast(F32))
                # kv-state partial: per pair p, lhsT = ek cols of the pair,
                # rhs = v cols of its quad (256 wide keeps float32r full-rate)
                for q in range(4):
                    kps = psum.tile([128, 512], F32, tag="mm")
                    for pp in range(2):
                        p = 2 * q + pp
                        nc.tensor.matmul(
                            kps[:, pp * 256:(pp + 1) * 256],
                            (ek[:, p * 128:(p + 1) * 128]),
                            (vv[:, q * 256:(q + 1) * 256]),
                            start=True, stop=True)
                    if tb == 0:
                        nc.vector.tensor_copy(kvacc[:, q * 512:(q + 1) * 512], kps)
                    else:
                        nc.vector.tensor_add(kvacc[:, q * 512:(q + 1) * 512],
                                             kvacc[:, q * 512:(q + 1) * 512], kps)

            # ---- Z readout (accumulated in zps during the tb loop) ----
            zrow = accp.tile([1, D], F32, tag="zrow")
            for half in range(2):
                sl = slice(half * 512, (half + 1) * 512)
                nc.scalar.copy(zrow[:, sl], zps[half][0:1, :])
            zdram = dram.tile([D], F32, tag="zd")
            nc.sync.dma_start(zdram[:].unsqueeze(0), zrow)

            # ---- stage compacted partial (kv | Z), AllReduce across seq pair
            # pair p -> cols [64p : 64p+64]; head A rows 0:64, head B 64:128
            stage = accp.tile([128, 520], F32, tag="stage")
            for p in range(NPAIR):
                q, pp = divmod(p, 2)
                colA = q * 512 + pp * 384    # head A cols inside kvacc
                nc.vector.tensor_copy(stage[0:64, 64 * p:64 * p + 64],
                                      kvacc[0:64, colA:colA + 64])
                nc.vector.tensor_copy(stage[64:128, 64 * p:64 * p + 64],
                                      kvacc[64:128, colA + 64:colA + 128])
            nc.sync.dma_start(stage[:, 512:520],
                             zdram[:].rearrange("(g p) -> p g", p=128))
            cin = dram.tile([128, 520], F32, tag="cin")
            cout = dram.tile([128, 520], F32, tag="cout")
            nc.sync.dma_start(cin, stage)
            nc.gpsimd.collective_compute(
                "AllReduce", mybir.AluOpType.add,
                replica_groups=[[0, 1], [2, 3], [4, 5], [6, 7]],
                ins=[cin[:].opt()], outs=[cout[:].opt()])
            kvred = accp.tile([128, 520], F32, tag="kvred")
            nc.sync.dma_start(kvred, cout)

            # ---- normalize kv state: rows d scaled by 1/Z[d] ----
            if debug:
                nc.sync.dma_start(dbg_kv[:], kvred)
            rz = accp.tile([128, NPAIR], F32, tag="rz")
            nc.vector.reciprocal(rz, kvred[:, 512:520])
            kv_sb = [kvsbp.tile([128, 128], F32R, tag=f"kv{p}", name=f"kv{p}") for p in range(NPAIR)]
            for p in range(NPAIR):
                # off-diagonal head-cross blocks must be exact zeros
                nc.vector.tensor_scalar_mul(
                    kv_sb[p][0:64, 64:128],
                    kvred[0:64, 64 * p:64 * p + 64], 0.0)
                nc.vector.tensor_scalar_mul(
                    kv_sb[p][64:128, 0:64],
                    kvred[64:128, 64 * p:64 * p + 64], 0.0)
                nc.vector.tensor_scalar_mul(
                    kv_sb[p][0:64, 0:64],
                    kvred[0:64, 64 * p:64 * p + 64],
                    rz[0:64, p:p + 1])
                nc.vector.tensor_scalar_mul(
                    kv_sb[p][64:128, 64:128],
                    kvred[64:128, 64 * p:64 * p + 64],
                    rz[64:128, p:p + 1])

            # ---- Phases 2-5 in two token halves (fits SBUF); the q sweep has
            # no dependency on the collective so it overlaps it naturally.
            wq_sb = [wpool.tile([128, D], F32R, tag=f"w{kc}", name=f"wq{kc}")
                     for kc in range(KC)]
            for kc in range(KC):
                nc.sync.dma_start(wq_sb[kc], wq_v[:, kc, :])
            wp_sb = [wpool.tile([128, D], F32R, tag=f"w{kc}", name=f"wp{kc}")
                     for kc in range(KC)]
            for kc in range(KC):
                nc.sync.dma_start(wp_sb[kc], wp_v[:, kc, :])
            TH = T // 2
            qtalls = []
            for th in range(2):
                qtall = qtpool.tile([128, NPAIR, TH], F32R, tag=f"qtall{th}",
                                    name=f"qtall_{th}")
                qtalls.append(qtall)
                for tb in range(TH // 128):
                    tbg = th * (TH // 128) + tb
                    xtile = xin.tile([128, KC, 128], F32R, tag="x")
                    nc.sync.dma_start(xtile, xt_v[:, :, tbg * 128:(tbg + 1) * 128])
                    eq = qpool.tile([128, D], F32R, tag="eq")
                    for half in range(2):
                        sl = slice(half * 512, (half + 1) * 512)
                        ps = psum.tile([128, 512], F32, tag="mm")
                        for kc in range(KC):
                            nc.tensor.matmul(ps, (xtile[:, kc, :]), (wq_sb[kc][:, sl]),
                                             start=(kc == 0), stop=(kc == KC - 1))
                        if with_bias:
                            nc.vector.tensor_add(ps, ps, bq_sb[:, sl])
                        nc.scalar.activation(eq[:, sl], ps, mybir.ActivationFunctionType.Exp)
                    sums = qpool.tile([128, NH], F32, tag="sums")
                    nc.vector.reduce_sum(sums, eq[:].rearrange("p (h e) -> p h e", e=HD),
                                         axis=mybir.AxisListType.X)
                    rfac = qpool.tile([128, NH], F32, tag="rfac")
                    nc.vector.reciprocal(rfac, sums)
                    nc.scalar.mul(rfac, rfac, SCALE)
                    # normalize per head on ACT (frees DVE, the co-bottleneck)
                    for h in range(NH):
                        nc.scalar.mul(eq[:, h * HD:(h + 1) * HD],
                                      eq[:, h * HD:(h + 1) * HD],
                                      rfac[:, h:h + 1])
                    for g4 in range(2):
                        tp = psum.tile([128, 512], F32R, tag="tr", bufs=2)
                        for j in range(4):
                            p = 4 * g4 + j
                            nc.tensor.transpose(tp[:, j * 128:(j + 1) * 128],
                                                eq[:, p * 128:(p + 1) * 128], ident)
                        nc.vector.tensor_copy(
                            qtall[:, 4 * g4:4 * g4 + 4, tb * 128:(tb + 1) * 128],
                            tp[:].rearrange("p (j t) -> p j t", j=4))

            for th in range(2):
                qtall = qtalls[th]
                # ---- attention output (e-major), written back into qT tiles
                for p in range(NPAIR):
                    for tcn in range(TH // 512):
                        sl = slice(tcn * 512, (tcn + 1) * 512)
                        ps = psum.tile([128, 512], F32, tag="mm")
                        nc.tensor.matmul(ps, (kv_sb[p]), (qtall[:, p, sl]),
                                         start=True, stop=True)
                        nc.vector.tensor_copy(qtall[:, p, sl], ps)

                # ---- output projection (token-major, natural store) ----
                for tb in range(TH // 128):
                    tbg = th * (TH // 128) + tb
                    for oc in range(2):
                        sl = slice(oc * 512, (oc + 1) * 512)
                        ps = psum.tile([128, 512], F32, tag="mm")
                        for kc in range(KC):
                            nc.tensor.matmul(
                                ps, (qtall[:, kc, tb * 128:(tb + 1) * 128]),
                                (wp_sb[kc][:, sl]),
                                start=(kc == 0), stop=(kc == KC - 1))
                        yt = youtp.tile([128, 512], F32, tag="y")
                        if with_bias:
                            nc.vector.tensor_add(yt, ps, bp_sb[:, sl])
                        else:
                            nc.vector.tensor_copy(yt, ps)
                        nc.sync.dma_start(y[tbg * 128:(tbg + 1) * 128, sl], yt)


    nc.compile()
    return nc


_NC = {}


def _get_nc(with_bias=True):
    if with_bias not in _NC:
        _NC[with_bias] = build_program(with_bias=with_bias)
    return _NC[with_bias]


def kernel(x, w_qkv, b_qkv, w_proj, b_proj):
    x = np.asarray(x, dtype=np.float32)
    w_qkv = np.asarray(w_qkv, dtype=np.float32)
    b_qkv = np.asarray(b_qkv, dtype=np.float32)
    w_proj = np.asarray(w_proj, dtype=np.float32)
    b_proj = np.asarray(b_proj, dtype=np.float32)

    bs, seqlen, dim = x.shape
    half = seqlen // 2

    wq = np.ascontiguousarray(w_qkv[0:D].T)
    wk = np.ascontiguousarray(w_qkv[D:2 * D].T)
    wv = np.ascontiguousarray(w_qkv[2 * D:3 * D].T)
    wp = np.ascontiguousarray(w_proj.T)
    bq, bk, bv = b_qkv[0:D], b_qkv[D:2 * D], b_qkv[2 * D:3 * D]

    cst = np.concatenate(
        [np.eye(128, dtype=np.float32),
         np.ones((128, 1), dtype=np.float32),
         np.zeros((128, 3), dtype=np.float32)], axis=1)

    in_maps = []
    for c in range(N_CORES):
        b, s = divmod(c, 2)
        chunk = np.ascontiguousarray(x[b, s * half:(s + 1) * half, :].T)
        in_maps.append({
            "xt": chunk, "wq": wq, "wk": wk, "wv": wv, "wp": wp,
            "bq": np.ascontiguousarray(bq), "bk": np.ascontiguousarray(bk),
            "bv": np.ascontiguousarray(bv), "bp": np.ascontiguousarray(b_proj),
            "cst": cst,
        })

    with_bias = bool(np.any(b_qkv)) or bool(np.any(b_proj))
    nc = _get_nc(with_bias)
    global _last_in_maps
    _last_in_maps = in_maps
    res = bass_utils.run_bass_kernel_spmd(nc, in_maps, core_ids=list(range(N_CORES)))

    out = np.empty((bs, seqlen, dim), dtype=np.float32)
    for c in range(N_CORES):
        b, s = divmod(c, 2)
        out[b, s * half:(s + 1) * half, :] = res.results[c]["y"]
    return out



# revision 17
# speedup vs baseline: 1.3069x; 1.3069x over previous
"""Trainium2 Bass kernel for EfficientAttention (linear attention block), v2.

Computation (per batch b, head h):
    qkv = x @ w_qkv.T (+ b_qkv)
    q = softmax(q, axis=head_dim) * head_dim**-0.5
    k = softmax(k, axis=seqlen)
    kv[d,e] = sum_s k[s,d] v[s,e]          (per-head 64x64 state)
    out[s,e] = sum_d q[s,d] kv[d,e]
    y = out @ w_proj.T (+ b_proj)

Sharding: 8 cores = (batch b = c//2, seq half = c%2); 2048 tokens per core,
all 16 heads. Cross-core coupling is only the kv state and the k-softmax
denominator Z -> one small AllReduce (pairs of cores) of [129, 1024] fp32.

v2 design vs v1 (523us):
- all matmuls in bf16 (fp32 "HIGH" PE mode streams ~1.9 Grow/s and is
  power-throttled; quantization error ~0.1% per tensor, budget 2e-2)
- M-fold: M[hd,j] = sum_e kv_norm[d,e] w_proj.T[e,j] computed once after
  the collective (8 matmuls), replacing the separate q@kv stage and its
  PSUM->SBUF copies; out-proj contracts q_T directly against M.
- kv state accumulated in PSUM across 4-tb blocks (pair-major group order
  to avoid interleaved accumulation groups per bank), folded to SBUF by
  DVE once per block.
- q normalization: one DVE broadcast-multiply (stride-0 AP) instead of 16
  scalar-engine ops per token block (scalar op overhead ~0.45us each).
- eq -> qT transposes moved off the PE onto the DMA XBAR (8x [128,128]
  bf16 transposes per token block, on otherwise-idle queues).
- x loaded once (bf16), kept in SBUF for both the k/v and q sweeps.
- y stores via one ACT copy [128,1024] + one DMA per token block.
"""

import sys

sys.path.insert(0, "/opt/trn_rl_repo")

import numpy as np
import ml_dtypes

import concourse.bacc as bacc
import concourse.bass as bass
import concourse.tile as tile
from concourse import mybir
from concourse import bass_utils

F32 = mybir.dt.float32
BF16 = mybir.dt.bfloat16

D = 1024          # model dim (= qkv contraction dim)
T = 2048          # tokens per core (one batch element's half sequence)
NH = 16           # heads
HD = 64           # head dim
NPAIR = 8         # head pairs (2 heads / 128 partitions)
KC = D // 128     # contraction chunks of 128
TB = T // 128     # token blocks of 128
SCALE = HD ** -0.5

N_CORES = 8


def bcast_part(ap, n=128):
    """View a [1, N] AP as [n, N] with partition step 0 (DMA broadcast)."""
    return bass.AP(tensor=ap.tensor, offset=ap.offset,
                   ap=[[0, n]] + list(ap.ap[1:]))


def build_program(with_bias=False):
    nc = bacc.Bacc("TRN2", target_bir_lowering=False, num_devices=N_CORES)

    xt = nc.dram_tensor("xt", [D, T], BF16, kind="ExternalInput")    # x chunk, transposed
    wq = nc.dram_tensor("wq", [D, D], BF16, kind="ExternalInput")    # w_q.T
    wk = nc.dram_tensor("wk", [D, D], BF16, kind="ExternalInput")    # w_k.T
    wv = nc.dram_tensor("wv", [D, D], BF16, kind="ExternalInput")    # w_v.T
    wp = nc.dram_tensor("wp", [D, D], BF16, kind="ExternalInput")    # w_proj.T
    onesd = nc.dram_tensor("onesd", [128, 1], BF16, kind="ExternalInput")
    maskd = nc.dram_tensor("maskd", [128, D], BF16, kind="ExternalInput")
    if with_bias:
        ebq = nc.dram_tensor("ebq", [D], F32, kind="ExternalInput")  # exp(b_q)
        by = nc.dram_tensor("by", [D], F32, kind="ExternalInput")    # folded out bias
    y = nc.dram_tensor("y", [T, D], F32, kind="ExternalOutput")

    xt_v = xt.rearrange("(kc p) t -> p kc t", p=128)
    wq_v = wq.rearrange("(kc p) f -> p kc f", p=128)
    wk_v = wk.rearrange("(kc p) f -> p kc f", p=128)
    wv_v = wv.rearrange("(kc p) f -> p kc f", p=128)
    wp_v = wp.rearrange("(kc p) f -> p kc f", p=128)

    with tile.TileContext(nc) as tc:
        with (
            tc.tile_pool(name="const", bufs=1) as const,
            tc.tile_pool(name="wpool", bufs=1) as wpool,
            tc.tile_pool(name="xpool", bufs=1) as xpool,
            tc.tile_pool(name="ekv", bufs=4) as ekv,
            tc.tile_pool(name="accp", bufs=1) as accp,
            tc.tile_pool(name="qpool", bufs=2) as qpool,
            tc.tile_pool(name="qtp", bufs=1) as qtp,
            tc.tile_pool(name="mpool", bufs=1) as mpool,
            tc.tile_pool(name="ytp", bufs=2) as ytp,
            tc.tile_pool(name="psum", bufs=1, space="PSUM") as psum,
            tc.tile_pool(name="dram", bufs=1, space="DRAM") as dram,
        ):
            ones_sb = const.tile([128, 1], BF16, tag="ones")
            nc.sync.dma_start(ones_sb, onesd[:])
            mask_sb = const.tile([128, D], BF16, tag="mask")
            nc.sync.dma_start(mask_sb, maskd[:])
            if with_bias:
                ebq_sb = const.tile([128, D], F32, tag="ebq")
                nc.gpsimd.dma_start(ebq_sb, bcast_part(ebq[:].unsqueeze(0)))
                by_sb = const.tile([128, D], F32, tag="by")
                nc.gpsimd.dma_start(by_sb, bcast_part(by[:].unsqueeze(0)))

            # weights: per-chunk tiles so matmuls can start before the full
            # matrix lands. k/v weights first (phase 1), then q, then proj.
            wk_sb = [wpool.tile([128, D], BF16, tag=f"wk{kc}", name=f"wk{kc}")
                     for kc in range(KC)]
            wv_sb = [wpool.tile([128, D], BF16, tag=f"wv{kc}", name=f"wv{kc}")
                     for kc in range(KC)]
            wq_sb = [wpool.tile([128, D], BF16, tag=f"wq{kc}", name=f"wq{kc}")
                     for kc in range(KC)]
            wp_sb = [wpool.tile([128, D], BF16, tag=f"wp{kc}", name=f"wp{kc}")
                     for kc in range(KC)]
            for kc in range(KC):
                nc.sync.dma_start(wk_sb[kc], wk_v[:, kc, :])
                nc.gpsimd.dma_start(wv_sb[kc], wv_v[:, kc, :])
            for kc in range(KC):
                nc.gpsimd.dma_start(wq_sb[kc], wq_v[:, kc, :])
                nc.gpsimd.dma_start(wp_sb[kc], wp_v[:, kc, :])

            xall = xpool.tile([128, KC, T], BF16, tag="xall")
            kvacc = accp.tile([128, D], F32, tag="kvacc")
            zrow = accp.tile([1, D], F32, tag="zrow")
            qtall = qtp.tile([128, KC, T], BF16, tag="qtall")

            # ---- Phase 1: k/v projections, exp(k), kv state + Z ----
            # PSUM budget (8 banks): proj tag [128,1024] x2 bufs = 4 banks,
            # kv [128,1024] = 2 banks, z 2x[1,512] = 2 banks.
            zps = [psum.tile([1, 512], F32, tag=f"z{h}", name=f"zps{h}", bufs=1)
                   for h in range(2)]
            kvps = psum.tile([128, D], F32, tag="kv", bufs=1)
            eks, vvs = [], []
            for tb in range(TB):
                tsl = slice(tb * 128, (tb + 1) * 128)
                nc.sync.dma_start(xall[:, :, tsl], xt_v[:, :, tsl])
                kps = psum.tile([128, D], F32, tag="proj", name="kps", bufs=2)
                for half in range(2):
                    sl = slice(half * 512, (half + 1) * 512)
                    for kc in range(KC):
                        nc.tensor.matmul(kps[:, sl], xall[:, kc, tsl],
                                         wk_sb[kc][:, sl],
                                         start=(kc == 0), stop=(kc == KC - 1))
                ek = ekv.tile([128, D], BF16, tag="ek")
                nc.scalar.activation(ek, kps, mybir.ActivationFunctionType.Exp)
                vps = psum.tile([128, D], F32, tag="proj", name="vps", bufs=2)
                for half in range(2):
                    sl = slice(half * 512, (half + 1) * 512)
                    for kc in range(KC):
                        nc.tensor.matmul(vps[:, sl], xall[:, kc, tsl],
                                         wv_sb[kc][:, sl],
                                         start=(kc == 0), stop=(kc == KC - 1))
                vv = ekv.tile([128, D], BF16, tag="vv")
                nc.scalar.copy(vv, vps)
                # Z partials: single accumulation group per bank across all
                # tbs (safe); ek must stay alive until its Z matmul runs.
                for half in range(2):
                    sl = slice(half * 512, (half + 1) * 512)
                    nc.tensor.matmul(zps[half], ones_sb, ek[:, sl],
                                     start=(tb == 0), stop=(tb == TB - 1))
                eks.append(ek)
                vvs.append(vv)
                if tb % 4 == 3:
                    # kv state for the last 4 tbs, pair-major so each PSUM
                    # bank sees its accumulation groups sequentially, never
                    # interleaved (interleaving corrupts, measured on HW).
                    for p in range(NPAIR):
                        csl = slice(p * 128, (p + 1) * 128)
                        for j in range(4):
                            nc.tensor.matmul(kvps[:, csl],
                                             vvs[j][:, csl], eks[j][:, csl],
                                             start=(j == 0), stop=(j == 3))
                    if tb == 3:
                        nc.vector.tensor_copy(kvacc, kvps)
                    else:
                        nc.vector.tensor_add(kvacc, kvacc, kvps)
                    eks, vvs = [], []

            for half in range(2):
                sl = slice(half * 512, (half + 1) * 512)
                nc.scalar.copy(zrow[:, sl], zps[half])

            # ---- AllReduce (kv | Z) across the sequence pair ----
            cin = dram.tile([129, D], F32, tag="cin")
            cout = dram.tile([129, D], F32, tag="cout")
            nc.sync.dma_start(cin[0:128, :], kvacc)
            nc.sync.dma_start(cin[128:129, :], zrow)
            nc.gpsimd.collective_compute(
                "AllReduce", mybir.AluOpType.add,
                replica_groups=[[0, 1], [2, 3], [4, 5], [6, 7]],
                ins=[cin[:].opt()], outs=[cout[:].opt()])
            kvred = accp.tile([128, D], F32, tag="kvred")
            nc.sync.dma_start(kvred, cout[0:128, :])
            # Z comes back as columns: partition r, col g  <-  Z[128 g + r]
            zcols = accp.tile([128, KC], F32, tag="zcols")
            nc.sync.dma_start(
                zcols, cout[128:129, :].rearrange("o (g p) -> (o p) g", p=128))

            # ---- Phase 2: q sweep (overlaps the collective) ----
            for tb in range(TB):
                tsl = slice(tb * 128, (tb + 1) * 128)
                qps = psum.tile([128, D], F32, tag="proj", name="qps", bufs=2)
                for half in range(2):
                    sl = slice(half * 512, (half + 1) * 512)
                    for kc in range(KC):
                        nc.tensor.matmul(qps[:, sl], xall[:, kc, tsl],
                                         wq_sb[kc][:, sl],
                                         start=(kc == 0), stop=(kc == KC - 1))
                eq = qpool.tile([128, D], BF16, tag="eq")
                nc.scalar.activation(eq, qps, mybir.ActivationFunctionType.Exp)
                if with_bias:
                    nc.vector.tensor_mul(eq, eq, ebq_sb)
                sums = qpool.tile([128, NH], F32, tag="sums")
                nc.vector.reduce_sum(sums, eq[:].rearrange("p (h e) -> p h e", e=HD),
                                     axis=mybir.AxisListType.X)
                rfac = qpool.tile([128, NH], F32, tag="rfac")
                nc.vector.reciprocal(rfac, sums)
                rfs = qpool.tile([128, NH], BF16, tag="rfs")
                nc.vector.tensor_scalar_mul(rfs, rfac, SCALE)
                # normalize: eq[p, h, e] *= rfs[p, h] via stride-0 broadcast
                rfs_ap = rfs[:]
                rfs_b = bass.AP(tensor=rfs_ap.tensor, offset=rfs_ap.offset,
                                ap=list(rfs_ap.ap) + [[0, HD]])
                eq_v = eq[:].rearrange("p (h e) -> p h e", e=HD)
                nc.vector.tensor_tensor(eq_v, eq_v, rfs_b, op=mybir.AluOpType.mult)
                # qT via DMA XBAR transpose, alternating the two hwdge queues
                for p in range(NPAIR):
                    eng = nc.sync if p % 2 == 0 else nc.scalar
                    eng.dma_start(qtall[:, p, tsl],
                                  eq[:, p * 128:(p + 1) * 128], transpose=True)

            # ---- mask kv, fold with w_proj into M, 1/Z as M's row scale ----
            rzs = accp.tile([128, KC], F32, tag="rzs")
            nc.vector.reciprocal(rzs, zcols)
            kvsb = accp.tile([128, D], BF16, tag="kvsb")
            nc.vector.tensor_mul(kvsb, kvred, mask_sb)
            m_sb = mpool.tile([128, KC, D], BF16, tag="m_sb")
            for p in range(NPAIR):
                mps = psum.tile([128, D], F32, tag="kv", name="mps")
                for half in range(2):
                    sl = slice(half * 512, (half + 1) * 512)
                    nc.tensor.matmul(mps[:, sl], kvsb[:, p * 128:(p + 1) * 128],
                                     wp_sb[p][:, sl], start=True, stop=True)
                # M_p rows are d-local of pair p: scale by 1/Z[128p + r]
                nc.scalar.mul(m_sb[:, p, :], mps, rzs[:, p:p + 1])

            # ---- Phase 4: out projection y = qT.T @ M ----
            for tb in range(TB):
                tsl = slice(tb * 128, (tb + 1) * 128)
                yps = psum.tile([128, D], F32, tag="proj", name="yps", bufs=2)
                for half in range(2):
                    sl = slice(half * 512, (half + 1) * 512)
                    for c in range(KC):
                        nc.tensor.matmul(yps[:, sl], qtall[:, c, tsl],
                                         m_sb[:, c, sl],
                                         start=(c == 0), stop=(c == KC - 1))
                yt = ytp.tile([128, D], F32, tag="yt")
                if with_bias:
                    nc.vector.tensor_add(yt, yps, by_sb)
                else:
                    nc.scalar.copy(yt, yps)
                nc.gpsimd.dma_start(y[tsl, :], yt)

    nc.compile()
    return nc


_NC = {}


def _get_nc(with_bias=False):
    if with_bias not in _NC:
        _NC[with_bias] = build_program(with_bias=with_bias)
    return _NC[with_bias]


def kernel(x, w_qkv, b_qkv, w_proj, b_proj):
    x = np.asarray(x, dtype=np.float32)
    w_qkv = np.asarray(w_qkv, dtype=np.float32)
    b_qkv = np.asarray(b_qkv, dtype=np.float32)
    w_proj = np.asarray(w_proj, dtype=np.float32)
    b_proj = np.asarray(b_proj, dtype=np.float32)

    bs, seqlen, dim = x.shape
    half = seqlen // 2
    bf = ml_dtypes.bfloat16

    wqm = np.ascontiguousarray(w_qkv[0:D].T.astype(bf))
    wkm = np.ascontiguousarray(w_qkv[D:2 * D].T.astype(bf))
    wvm = np.ascontiguousarray(w_qkv[2 * D:3 * D].T.astype(bf))
    wpm = np.ascontiguousarray(w_proj.T.astype(bf))
    bq, bv = b_qkv[0:D], b_qkv[2 * D:3 * D]

    ones = np.ones((128, 1), dtype=bf)
    # mask[e_local, d_local] per 128-col pair block: head-diagonal blocks
    blk = np.zeros((128, 128), np.float32)
    blk[0:64, 0:64] = 1.0
    blk[64:128, 64:128] = 1.0
    maskc = np.ascontiguousarray(np.tile(blk, (1, KC)).astype(bf))

    with_bias = bool(np.any(b_qkv)) or bool(np.any(b_proj))

    in_maps = []
    for c in range(N_CORES):
        b, s = divmod(c, 2)
        chunk = np.ascontiguousarray(x[b, s * half:(s + 1) * half, :].T.astype(bf))
        im = {"xt": chunk, "wq": wqm, "wk": wkm, "wv": wvm, "wp": wpm,
              "onesd": ones, "maskd": maskc}
        if with_bias:
            im["ebq"] = np.exp(bq).astype(np.float32)
            im["by"] = (SCALE * (w_proj @ bv) + b_proj).astype(np.float32)
        in_maps.append(im)

    nc = _get_nc(with_bias)
    global _last_in_maps
    _last_in_maps = in_maps
    res = bass_utils.run_bass_kernel_spmd(nc, in_maps, core_ids=list(range(N_CORES)))

    out = np.empty((bs, seqlen, dim), dtype=np.float32)
    for c in range(N_CORES):
        b, s = divmod(c, 2)
        out[b, s * half:(s + 1) * half, :] = res.results[c]["y"]
    return out


# revision 24
# speedup vs baseline: 1.6328x; 1.2494x over previous
"""Trainium2 Bass kernel for EfficientAttention (linear attention block), v2.

Computation (per batch b, head h):
    qkv = x @ w_qkv.T (+ b_qkv)
    q = softmax(q, axis=head_dim) * head_dim**-0.5
    k = softmax(k, axis=seqlen)
    kv[d,e] = sum_s k[s,d] v[s,e]          (per-head 64x64 state)
    out[s,e] = sum_d q[s,d] kv[d,e]
    y = out @ w_proj.T (+ b_proj)

Sharding: 8 cores = (batch b = c//2, seq half = c%2); 2048 tokens per core,
all 16 heads. Cross-core coupling is only the kv state and the k-softmax
denominator Z -> one small AllReduce (pairs of cores) of [129, 1024] fp32.

v2 design vs v1 (523us):
- all matmuls in bf16 (fp32 "HIGH" PE mode streams ~1.9 Grow/s and is
  power-throttled; quantization error ~0.1% per tensor, budget 2e-2)
- M-fold: M[hd,j] = sum_e kv_norm[d,e] w_proj.T[e,j] computed once after
  the collective (8 matmuls), replacing the separate q@kv stage and its
  PSUM->SBUF copies; out-proj contracts q_T directly against M.
- kv state accumulated in PSUM across 4-tb blocks (pair-major group order
  to avoid interleaved accumulation groups per bank), folded to SBUF by
  DVE once per block.
- q normalization: one DVE broadcast-multiply (stride-0 AP) instead of 16
  scalar-engine ops per token block (scalar op overhead ~0.45us each).
- eq -> qT transposes moved off the PE onto the DMA XBAR (8x [128,128]
  bf16 transposes per token block, on otherwise-idle queues).
- x loaded once (bf16), kept in SBUF for both the k/v and q sweeps.
- y stores via one ACT copy [128,1024] + one DMA per token block.
"""

import sys

sys.path.insert(0, "/opt/trn_rl_repo")

import numpy as np
import ml_dtypes

import concourse.bacc as bacc
import concourse.bass as bass
import concourse.tile as tile
from concourse import mybir
from concourse import bass_utils

F32 = mybir.dt.float32
BF16 = mybir.dt.bfloat16

D = 1024          # model dim (= qkv contraction dim)
T = 2048          # tokens per core (one batch element's half sequence)
NH = 16           # heads
HD = 64           # head dim
NPAIR = 8         # head pairs (2 heads / 128 partitions)
KC = D // 128     # contraction chunks of 128
TB = T // 128     # token blocks of 128
SCALE = HD ** -0.5

N_CORES = 8


def bcast_part(ap, n=128):
    """View a [1, N] AP as [n, N] with partition step 0 (DMA broadcast)."""
    return bass.AP(tensor=ap.tensor, offset=ap.offset,
                   ap=[[0, n]] + list(ap.ap[1:]))


def build_program(with_bias=False):
    nc = bacc.Bacc("TRN2", target_bir_lowering=False, num_devices=N_CORES)

    xt = nc.dram_tensor("xt", [D, T], BF16, kind="ExternalInput")    # x chunk, transposed
    wq = nc.dram_tensor("wq", [D, D], BF16, kind="ExternalInput")    # w_q.T
    wk = nc.dram_tensor("wk", [D, D], BF16, kind="ExternalInput")    # w_k.T
    wv = nc.dram_tensor("wv", [D, D], BF16, kind="ExternalInput")    # w_v.T
    wp = nc.dram_tensor("wp", [D, D], BF16, kind="ExternalInput")    # w_proj.T
    onesd = nc.dram_tensor("onesd", [128, 1], BF16, kind="ExternalInput")
    maskd = nc.dram_tensor("maskd", [128, D], BF16, kind="ExternalInput")
    identd = nc.dram_tensor("identd", [128, 128], BF16, kind="ExternalInput")
    if with_bias:
        ebq = nc.dram_tensor("ebq", [D], F32, kind="ExternalInput")  # exp(b_q)
        by = nc.dram_tensor("by", [D], F32, kind="ExternalInput")    # folded out bias
    y = nc.dram_tensor("y", [T, D], F32, kind="ExternalOutput")

    xt_v = xt.rearrange("(kc p) t -> p kc t", p=128)
    wq_v = wq.rearrange("(kc p) f -> p kc f", p=128)
    wk_v = wk.rearrange("(kc p) f -> p kc f", p=128)
    wv_v = wv.rearrange("(kc p) f -> p kc f", p=128)
    wp_v = wp.rearrange("(kc p) f -> p kc f", p=128)

    with tile.TileContext(nc) as tc:
        with (
            tc.tile_pool(name="const", bufs=1) as const,
            tc.tile_pool(name="wpool", bufs=1) as wpool,
            tc.tile_pool(name="xpool", bufs=1) as xpool,
            tc.tile_pool(name="ekv", bufs=4) as ekv,
            tc.tile_pool(name="accp", bufs=1) as accp,
            tc.tile_pool(name="qpool", bufs=2) as qpool,
            tc.tile_pool(name="qtp", bufs=1) as qtp,
            tc.tile_pool(name="mpool", bufs=1) as mpool,
            tc.tile_pool(name="ytp", bufs=2) as ytp,
            tc.tile_pool(name="psum", bufs=1, space="PSUM") as psum,
            tc.tile_pool(name="dram", bufs=1, space="DRAM") as dram,
        ):
            # x for the first token blocks goes first on the sync queue so the
            # first k-projection can start as early as possible.
            xall = xpool.tile([128, KC, T], BF16, tag="xall")
            for tb in range(2):
                tsl = slice(tb * 128, (tb + 1) * 128)
                nc.sync.dma_start(xall[:, :, tsl], xt_v[:, :, tsl])
            ones_sb = const.tile([128, 1], BF16, tag="ones")
            nc.gpsimd.dma_start(ones_sb, onesd[:])
            mask_sb = const.tile([128, D], BF16, tag="mask")
            nc.gpsimd.dma_start(mask_sb, maskd[:])
            ident_sb = const.tile([128, 128], BF16, tag="ident")
            nc.gpsimd.dma_start(ident_sb, identd[:])
            if with_bias:
                ebq_sb = const.tile([128, D], F32, tag="ebq")
                nc.gpsimd.dma_start(ebq_sb, bcast_part(ebq[:].unsqueeze(0)))
                by_sb = const.tile([128, D], F32, tag="by")
                nc.gpsimd.dma_start(by_sb, bcast_part(by[:].unsqueeze(0)))

            # weights: per-chunk tiles so matmuls can start before the full
            # matrix lands. k/v weights first (phase 1), then q, then proj.
            wk_sb = [wpool.tile([128, D], BF16, tag=f"wk{kc}", name=f"wk{kc}")
                     for kc in range(KC)]
            wv_sb = [wpool.tile([128, D], BF16, tag=f"wv{kc}", name=f"wv{kc}")
                     for kc in range(KC)]
            wq_sb = [wpool.tile([128, D], BF16, tag=f"wq{kc}", name=f"wq{kc}")
                     for kc in range(KC)]
            wp_sb = [wpool.tile([128, D], BF16, tag=f"wp{kc}", name=f"wp{kc}")
                     for kc in range(KC)]
            for kc in range(KC):
                nc.sync.dma_start(wk_sb[kc], wk_v[:, kc, :])
                nc.gpsimd.dma_start(wv_sb[kc], wv_v[:, kc, :])
            for kc in range(KC):
                nc.gpsimd.dma_start(wq_sb[kc], wq_v[:, kc, :])
                nc.gpsimd.dma_start(wp_sb[kc], wp_v[:, kc, :])

            kvacc = accp.tile([128, D], F32, tag="kvacc")
            zrow = accp.tile([1, D], F32, tag="zrow")
            qtall = qtp.tile([128, KC, T], BF16, tag="qtall")

            # ---- Phase 1: k/v projections, exp(k), kv state + Z ----
            # PSUM budget (8 banks): proj tag [128,1024] x2 bufs = 4 banks,
            # kv [128,1024] = 2 banks, z 2x[1,512] = 2 banks.
            zps = [psum.tile([1, 512], F32, tag=f"z{h}", name=f"zps{h}", bufs=1)
                   for h in range(2)]
            kvps = psum.tile([128, D], F32, tag="kv", bufs=1)
            eks, vvs = [], []
            for tb in range(TB):
                tsl = slice(tb * 128, (tb + 1) * 128)
                if tb >= 2:
                    nc.sync.dma_start(xall[:, :, tsl], xt_v[:, :, tsl])
                kps = psum.tile([128, D], F32, tag="proj", name="kps", bufs=2)
                for half in range(2):
                    sl = slice(half * 512, (half + 1) * 512)
                    for kc in range(KC):
                        nc.tensor.matmul(kps[:, sl], xall[:, kc, tsl],
                                         wk_sb[kc][:, sl],
                                         start=(kc == 0), stop=(kc == KC - 1))
                ek = ekv.tile([128, D], BF16, tag="ek")
                nc.scalar.activation(ek, kps, mybir.ActivationFunctionType.Exp)
                vps = psum.tile([128, D], F32, tag="proj", name="vps", bufs=2)
                for half in range(2):
                    sl = slice(half * 512, (half + 1) * 512)
                    for kc in range(KC):
                        nc.tensor.matmul(vps[:, sl], xall[:, kc, tsl],
                                         wv_sb[kc][:, sl],
                                         start=(kc == 0), stop=(kc == KC - 1))
                vv = ekv.tile([128, D], BF16, tag="vv")
                nc.scalar.copy(vv, vps)
                # Z partials: single accumulation group per bank across all
                # tbs (safe); ek must stay alive until its Z matmul runs.
                for half in range(2):
                    sl = slice(half * 512, (half + 1) * 512)
                    nc.tensor.matmul(zps[half], ones_sb, ek[:, sl],
                                     start=(tb == 0), stop=(tb == TB - 1))
                eks.append(ek)
                vvs.append(vv)
                if tb % 4 == 3:
                    # kv state for the last 4 tbs, pair-major so each PSUM
                    # bank sees its accumulation groups sequentially, never
                    # interleaved (interleaving corrupts, measured on HW).
                    for p in range(NPAIR):
                        csl = slice(p * 128, (p + 1) * 128)
                        for j in range(4):
                            nc.tensor.matmul(kvps[:, csl],
                                             vvs[j][:, csl], eks[j][:, csl],
                                             start=(j == 0), stop=(j == 3))
                    if tb == 3:
                        nc.vector.tensor_copy(kvacc, kvps)
                    else:
                        nc.vector.tensor_add(kvacc, kvacc, kvps)
                    eks, vvs = [], []

            for half in range(2):
                sl = slice(half * 512, (half + 1) * 512)
                nc.scalar.copy(zrow[:, sl], zps[half])

            # ---- AllReduce (kv | Z) across the sequence pair ----
            cin = dram.tile([129, D], F32, tag="cin")
            cout = dram.tile([129, D], F32, tag="cout")
            nc.sync.dma_start(cin[0:128, :], kvacc)
            nc.sync.dma_start(cin[128:129, :], zrow)
            nc.gpsimd.collective_compute(
                "AllReduce", mybir.AluOpType.add,
                replica_groups=[[0, 1], [2, 3], [4, 5], [6, 7]],
                ins=[cin[:].opt()], outs=[cout[:].opt()])
            kvred = accp.tile([128, D], F32, tag="kvred")
            nc.sync.dma_start(kvred, cout[0:128, :])
            # Z comes back as columns: partition r, col g  <-  Z[128 g + r]
            zcols = accp.tile([128, KC], F32, tag="zcols")
            nc.sync.dma_start(
                zcols, cout[128:129, :].rearrange("o (g p) -> (o p) g", p=128))

            # ---- Phase 2: q sweep (overlaps the collective) ----
            for tb in range(TB):
                tsl = slice(tb * 128, (tb + 1) * 128)
                qps = psum.tile([128, D], F32, tag="proj", name="qps", bufs=2)
                for half in range(2):
                    sl = slice(half * 512, (half + 1) * 512)
                    for kc in range(KC):
                        nc.tensor.matmul(qps[:, sl], xall[:, kc, tsl],
                                         wq_sb[kc][:, sl],
                                         start=(kc == 0), stop=(kc == KC - 1))
                eq = qpool.tile([128, D], BF16, tag="eq")
                nc.scalar.activation(eq, qps, mybir.ActivationFunctionType.Exp)
                if with_bias:
                    nc.vector.tensor_mul(eq, eq, ebq_sb)
                sums = qpool.tile([128, NH], F32, tag="sums")
                nc.vector.reduce_sum(sums, eq[:].rearrange("p (h e) -> p h e", e=HD),
                                     axis=mybir.AxisListType.X)
                rfac = qpool.tile([128, NH], F32, tag="rfac")
                nc.vector.reciprocal(rfac, sums)
                rfs = qpool.tile([128, NH], BF16, tag="rfs")
                nc.vector.tensor_scalar_mul(rfs, rfac, SCALE)
                # normalize: eq[p, h, e] *= rfs[p, h] via stride-0 broadcast
                rfs_ap = rfs[:]
                rfs_b = bass.AP(tensor=rfs_ap.tensor, offset=rfs_ap.offset,
                                ap=list(rfs_ap.ap) + [[0, HD]])
                eq_v = eq[:].rearrange("p (h e) -> p h e", e=HD)
                nc.vector.tensor_tensor(eq_v, eq_v, rfs_b, op=mybir.AluOpType.mult)
                # qT via PE transposes into a 1-bank bf16 PSUM tile (the Z
                # banks are dead after phase 1; alternate them as 2 buffers),
                # then one strided DVE copy out to qtall.
                ztag = f"z{tb % 2}"
                tp = psum.tile([128, NPAIR, 128], BF16, tag=ztag, name="tp", bufs=1)
                for p in range(NPAIR):
                    nc.tensor.transpose(tp[:, p, :], eq[:, p * 128:(p + 1) * 128],
                                        ident_sb)
                nc.vector.tensor_copy(qtall[:, :, tsl], tp)

            # ---- mask kv, fold with w_proj into M, 1/Z as M's row scale ----
            rzs = accp.tile([128, KC], F32, tag="rzs")
            nc.vector.reciprocal(rzs, zcols)
            kvsb = accp.tile([128, D], BF16, tag="kvsb")
            nc.vector.tensor_mul(kvsb, kvred, mask_sb)
            m_sb = mpool.tile([128, KC, D], BF16, tag="m_sb")
            for p in range(NPAIR):
                mps = psum.tile([128, D], F32, tag="kv", name="mps")
                for half in range(2):
                    sl = slice(half * 512, (half + 1) * 512)
                    nc.tensor.matmul(mps[:, sl], kvsb[:, p * 128:(p + 1) * 128],
                                     wp_sb[p][:, sl], start=True, stop=True)
                # M_p rows are d-local of pair p: scale by 1/Z[128p + r]
                nc.scalar.mul(m_sb[:, p, :], mps, rzs[:, p:p + 1])

            # ---- Phase 4: out projection y = qT.T @ M ----
            for tb in range(TB):
                tsl = slice(tb * 128, (tb + 1) * 128)
                yps = psum.tile([128, D], F32, tag="proj", name="yps", bufs=2)
                for half in range(2):
                    sl = slice(half * 512, (half + 1) * 512)
                    for c in range(KC):
                        nc.tensor.matmul(yps[:, sl], qtall[:, c, tsl],
                                         m_sb[:, c, sl],
                                         start=(c == 0), stop=(c == KC - 1))
                yt = ytp.tile([128, D], F32, tag="yt")
                if with_bias:
                    nc.vector.tensor_add(yt, yps, by_sb)
                else:
                    nc.scalar.copy(yt, yps)
                nc.gpsimd.dma_start(y[tsl, :], yt)

    nc.compile()
    return nc


_NC = {}


def _get_nc(with_bias=False):
    if with_bias not in _NC:
        _NC[with_bias] = build_program(with_bias=with_bias)
    return _NC[with_bias]


def kernel(x, w_qkv, b_qkv, w_proj, b_proj):
    x = np.asarray(x, dtype=np.float32)
    w_qkv = np.asarray(w_qkv, dtype=np.float32)
    b_qkv = np.asarray(b_qkv, dtype=np.float32)
    w_proj = np.asarray(w_proj, dtype=np.float32)
    b_proj = np.asarray(b_proj, dtype=np.float32)

    bs, seqlen, dim = x.shape
    half = seqlen // 2
    bf = ml_dtypes.bfloat16

    wqm = np.ascontiguousarray(w_qkv[0:D].T.astype(bf))
    wkm = np.ascontiguousarray(w_qkv[D:2 * D].T.astype(bf))
    wvm = np.ascontiguousarray(w_qkv[2 * D:3 * D].T.astype(bf))
    wpm = np.ascontiguousarray(w_proj.T.astype(bf))
    bq, bv = b_qkv[0:D], b_qkv[2 * D:3 * D]

    ones = np.ones((128, 1), dtype=bf)
    ident = np.eye(128, dtype=bf)
    # mask[e_local, d_local] per 128-col pair block: head-diagonal blocks
    blk = np.zeros((128, 128), np.float32)
    blk[0:64, 0:64] = 1.0
    blk[64:128, 64:128] = 1.0
    maskc = np.ascontiguousarray(np.tile(blk, (1, KC)).astype(bf))

    with_bias = bool(np.any(b_qkv)) or bool(np.any(b_proj))

    in_maps = []
    for c in range(N_CORES):
        b, s = divmod(c, 2)
        chunk = np.ascontiguousarray(x[b, s * half:(s + 1) * half, :].T.astype(bf))
        im = {"xt": chunk, "wq": wqm, "wk": wkm, "wv": wvm, "wp": wpm,
              "onesd": ones, "maskd": maskc, "identd": ident}
        if with_bias:
            im["ebq"] = np.exp(bq).astype(np.float32)
            im["by"] = (SCALE * (w_proj @ bv) + b_proj).astype(np.float32)
        in_maps.append(im)

    nc = _get_nc(with_bias)
    global _last_in_maps
    _last_in_maps = in_maps
    res = bass_utils.run_bass_kernel_spmd(nc, in_maps, core_ids=list(range(N_CORES)))

    out = np.empty((bs, seqlen, dim), dtype=np.float32)
    for c in range(N_CORES):
        b, s = divmod(c, 2)
        out[b, s * half:(s + 1) * half, :] = res.results[c]["y"]
    return out


# revision 37
# speedup vs baseline: 1.6650x; 1.0197x over previous
"""Trainium2 Bass kernel for EfficientAttention (linear attention block), v2.

Computation (per batch b, head h):
    qkv = x @ w_qkv.T (+ b_qkv)
    q = softmax(q, axis=head_dim) * head_dim**-0.5
    k = softmax(k, axis=seqlen)
    kv[d,e] = sum_s k[s,d] v[s,e]          (per-head 64x64 state)
    out[s,e] = sum_d q[s,d] kv[d,e]
    y = out @ w_proj.T (+ b_proj)

Sharding: 8 cores = (batch b = c//2, seq half = c%2); 2048 tokens per core,
all 16 heads. Cross-core coupling is only the kv state and the k-softmax
denominator Z -> one small AllReduce (pairs of cores) of [129, 1024] fp32.

v2 design vs v1 (523us):
- all matmuls in bf16 (fp32 "HIGH" PE mode streams ~1.9 Grow/s and is
  power-throttled; quantization error ~0.1% per tensor, budget 2e-2)
- M-fold: M[hd,j] = sum_e kv_norm[d,e] w_proj.T[e,j] computed once after
  the collective (8 matmuls), replacing the separate q@kv stage and its
  PSUM->SBUF copies; out-proj contracts q_T directly against M.
- kv state accumulated in PSUM across 4-tb blocks (pair-major group order
  to avoid interleaved accumulation groups per bank), folded to SBUF by
  DVE once per block.
- q normalization: one DVE broadcast-multiply (stride-0 AP) instead of 16
  scalar-engine ops per token block (scalar op overhead ~0.45us each).
- eq -> qT transposes moved off the PE onto the DMA XBAR (8x [128,128]
  bf16 transposes per token block, on otherwise-idle queues).
- x loaded once (bf16), kept in SBUF for both the k/v and q sweeps.
- y stores via one ACT copy [128,1024] + one DMA per token block.
"""

import sys

sys.path.insert(0, "/opt/trn_rl_repo")

import numpy as np
import ml_dtypes

import concourse.bacc as bacc
import concourse.bass as bass
import concourse.tile as tile
from concourse import mybir
from concourse import bass_utils

F32 = mybir.dt.float32
BF16 = mybir.dt.bfloat16

D = 1024          # model dim (= qkv contraction dim)
T = 2048          # tokens per core (one batch element's half sequence)
NH = 16           # heads
HD = 64           # head dim
NPAIR = 8         # head pairs (2 heads / 128 partitions)
KC = D // 128     # contraction chunks of 128
TB = T // 128     # token blocks of 128
SCALE = HD ** -0.5

N_CORES = 8


def bcast_part(ap, n=128):
    """View a [1, N] AP as [n, N] with partition step 0 (DMA broadcast)."""
    return bass.AP(tensor=ap.tensor, offset=ap.offset,
                   ap=[[0, n]] + list(ap.ap[1:]))


def build_program(with_bias=False):
    nc = bacc.Bacc("TRN2", target_bir_lowering=False, num_devices=N_CORES)

    xt = nc.dram_tensor("xt", [D, T], BF16, kind="ExternalInput")    # x chunk, transposed
    wq = nc.dram_tensor("wq", [D, D], BF16, kind="ExternalInput")    # w_q.T
    wk = nc.dram_tensor("wk", [D, D], BF16, kind="ExternalInput")    # w_k.T
    wv = nc.dram_tensor("wv", [D, D], BF16, kind="ExternalInput")    # w_v.T
    wp = nc.dram_tensor("wp", [D, D], BF16, kind="ExternalInput")    # w_proj.T
    maskd = nc.dram_tensor("maskd", [128, D], BF16, kind="ExternalInput")
    identd = nc.dram_tensor("identd", [128, 128], BF16, kind="ExternalInput")
    if with_bias:
        ebq = nc.dram_tensor("ebq", [D], F32, kind="ExternalInput")  # exp(b_q)
        by = nc.dram_tensor("by", [D], F32, kind="ExternalInput")    # folded out bias
    y = nc.dram_tensor("y", [T, D], F32, kind="ExternalOutput")

    xt_v = xt.rearrange("(kc p) t -> p kc t", p=128)
    wq_v = wq.rearrange("(kc p) f -> p kc f", p=128)
    wk_v = wk.rearrange("(kc p) f -> p kc f", p=128)
    wv_v = wv.rearrange("(kc p) f -> p kc f", p=128)
    wp_v = wp.rearrange("(kc p) f -> p kc f", p=128)

    with tile.TileContext(nc) as tc:
        with (
            tc.tile_pool(name="const", bufs=1) as const,
            tc.tile_pool(name="wpool", bufs=1) as wpool,
            tc.tile_pool(name="xpool", bufs=1) as xpool,
            tc.tile_pool(name="ekv", bufs=4) as ekv,
            tc.tile_pool(name="accp", bufs=1) as accp,
            tc.tile_pool(name="qpool", bufs=2) as qpool,
            tc.tile_pool(name="qtp", bufs=1) as qtp,
            tc.tile_pool(name="mpool", bufs=1) as mpool,
            tc.tile_pool(name="ytp", bufs=2) as ytp,
            tc.tile_pool(name="dram", bufs=1, space="DRAM") as dram,
        ):
            # x for the first token blocks goes first on the sync queue so the
            # first k-projection can start as early as possible.
            xall = xpool.tile([128, KC, T], BF16, tag="xall")
            for tb in range(2):
                tsl = slice(tb * 128, (tb + 1) * 128)
                nc.sync.dma_start(xall[:, :, tsl], xt_v[:, :, tsl])
            mask_sb = const.tile([128, D], BF16, tag="mask")
            nc.gpsimd.dma_start(mask_sb, maskd[:])
            ident_sb = const.tile([128, 128], BF16, tag="ident")
            nc.gpsimd.dma_start(ident_sb, identd[:])
            if with_bias:
                ebq_sb = const.tile([128, D], F32, tag="ebq")
                nc.gpsimd.dma_start(ebq_sb, bcast_part(ebq[:].unsqueeze(0)))
                by_sb = const.tile([128, D], F32, tag="by")
                nc.gpsimd.dma_start(by_sb, bcast_part(by[:].unsqueeze(0)))

            # weights: per-chunk tiles so matmuls can start before the full
            # matrix lands. k/v weights first (phase 1), then q, then proj.
            wk_sb = [wpool.tile([128, D], BF16, tag=f"wk{kc}", name=f"wk{kc}")
                     for kc in range(KC)]
            wv_sb = [wpool.tile([128, D], BF16, tag=f"wv{kc}", name=f"wv{kc}")
                     for kc in range(KC)]
            wq_sb = [wpool.tile([128, D], BF16, tag=f"wq{kc}", name=f"wq{kc}")
                     for kc in range(KC)]
            wp_sb = [wpool.tile([128, D], BF16, tag=f"wp{kc}", name=f"wp{kc}")
                     for kc in range(KC)]
            for kc in range(KC):
                nc.sync.dma_start(wk_sb[kc], wk_v[:, kc, :])
                nc.gpsimd.dma_start(wv_sb[kc], wv_v[:, kc, :])
            for kc in range(KC):
                nc.gpsimd.dma_start(wq_sb[kc], wq_v[:, kc, :])
                nc.gpsimd.dma_start(wp_sb[kc], wp_v[:, kc, :])

            kvacc = accp.tile([128, D], F32, tag="kvacc")
            zacc = accp.tile([128, KC], F32, tag="zacc")
            qtall = qtp.tile([128, KC, T], BF16, tag="qtall")

            # ---- Phase 1: k/v projections, exp(k), kv state + Z ----
            # kv is computed d-major (lhsT = ek pair), with a ones column
            # appended to v so each pair's matmul also produces its Z slice:
            # out[:, p, 0:128] = kv_p[d, e], out[:, p, 128] = Z[128p + d].
            # PSUM budget (8 banks): proj tag [128,1024] x2 bufs = 4 banks,
            # kv [128, 8, 256] = 4 banks (2 pairs per bank, groups ordered
            # pair-major so banks never see interleaved accumulation groups
            # -- interleaving corrupts, measured on HW).
            psum = tc.alloc_tile_pool(name="psum1", bufs=1, space="PSUM")
            kvps = psum.tile([128, NPAIR, 256], F32, tag="kv", bufs=1)
            vv_bufs = [ekv.tile([128, NPAIR, 132], BF16, tag=f"vv{j}",
                                name=f"vv{j}", bufs=1) for j in range(4)]
            for j in range(4):
                nc.gpsimd.memset(vv_bufs[j][:, :, 128:129], 1.0)
            eks = []
            for tb in range(TB):
                tsl = slice(tb * 128, (tb + 1) * 128)
                if tb >= 2:
                    nc.sync.dma_start(xall[:, :, tsl], xt_v[:, :, tsl])
                kps = psum.tile([128, D], F32, tag="proj", name="kps", bufs=2)
                for half in range(2):
                    sl = slice(half * 512, (half + 1) * 512)
                    for kc in range(KC):
                        nc.tensor.matmul(kps[:, sl], xall[:, kc, tsl],
                                         wk_sb[kc][:, sl],
                                         start=(kc == 0), stop=(kc == KC - 1))
                ek = ekv.tile([128, D], BF16, tag="ek")
                nc.scalar.activation(ek, kps, mybir.ActivationFunctionType.Exp)
                vps = psum.tile([128, D], F32, tag="proj", name="vps", bufs=2)
                for half in range(2):
                    sl = slice(half * 512, (half + 1) * 512)
                    for kc in range(KC):
                        nc.tensor.matmul(vps[:, sl], xall[:, kc, tsl],
                                         wv_sb[kc][:, sl],
                                         start=(kc == 0), stop=(kc == KC - 1))
                vv = vv_bufs[tb % 4]
                nc.scalar.copy(vv[:, :, 0:128],
                               vps[:].rearrange("p (g e) -> p g e", e=128))
                eks.append(ek)
                if tb % 4 == 3:
                    for p in range(NPAIR):
                        csl = slice(p * 128, (p + 1) * 128)
                        for j in range(4):
                            nc.tensor.matmul(kvps[:, p, 0:129],
                                             eks[j][:, csl],
                                             vv_bufs[j][:, p, 0:129],
                                             start=(j == 0), stop=(j == 3))
                    if tb == 3:
                        nc.vector.tensor_copy(kvacc[:].rearrange("p (g e) -> p g e", e=128),
                                              kvps[:, :, 0:128])
                        nc.vector.tensor_copy(zacc, kvps[:, :, 128])
                    else:
                        nc.vector.tensor_add(kvacc[:].rearrange("p (g e) -> p g e", e=128),
                                             kvacc[:].rearrange("p (g e) -> p g e", e=128),
                                             kvps[:, :, 0:128])
                        nc.vector.tensor_add(zacc, zacc, kvps[:, :, 128])
                    eks = []
            psum.release()

            # ---- AllReduce (kv | Z) across the sequence pair ----
            cin = dram.tile([128, D + KC], F32, tag="cin")
            cout = dram.tile([128, D + KC], F32, tag="cout")
            nc.sync.dma_start(cin[:, 0:D], kvacc)
            nc.sync.dma_start(cin[:, D:D + KC], zacc)
            nc.gpsimd.collective_compute(
                "AllReduce", mybir.AluOpType.add,
                replica_groups=[[0, 1], [2, 3], [4, 5], [6, 7]],
                ins=[cin[:].opt()], outs=[cout[:].opt()])
            kvred = accp.tile([128, D], F32, tag="kvred")
            nc.sync.dma_start(kvred, cout[:, 0:D])
            # Z columns: partition r, col g  <->  Z[128 g + r]
            zcols = accp.tile([128, KC], F32, tag="zcols")
            nc.sync.dma_start(zcols, cout[:, D:D + KC])

            # ---- Phase 2: q sweep (overlaps the collective) ----
            # PSUM: proj 2x2 banks, tp 2x1, ktp 1, mk 1 = 8.
            psum = tc.alloc_tile_pool(name="psum2", bufs=1, space="PSUM")
            for tb in range(TB):
                tsl = slice(tb * 128, (tb + 1) * 128)
                qps = psum.tile([128, D], F32, tag="proj", name="qps", bufs=2)
                for half in range(2):
                    sl = slice(half * 512, (half + 1) * 512)
                    for kc in range(KC):
                        nc.tensor.matmul(qps[:, sl], xall[:, kc, tsl],
                                         wq_sb[kc][:, sl],
                                         start=(kc == 0), stop=(kc == KC - 1))
                eq = qpool.tile([128, D], BF16, tag="eq")
                nc.scalar.activation(eq, qps, mybir.ActivationFunctionType.Exp)
                if with_bias:
                    nc.vector.tensor_mul(eq, eq, ebq_sb)
                sums = qpool.tile([128, NH], F32, tag="sums")
                nc.vector.reduce_sum(sums, eq[:].rearrange("p (h e) -> p h e", e=HD),
                                     axis=mybir.AxisListType.X)
                rfac = qpool.tile([128, NH], F32, tag="rfac")
                nc.vector.reciprocal(rfac, sums)
                rfs = qpool.tile([128, NH], BF16, tag="rfs")
                nc.vector.tensor_scalar_mul(rfs, rfac, SCALE)
                # normalize: eq[p, h, e] *= rfs[p, h] via stride-0 broadcast
                rfs_ap = rfs[:]
                rfs_b = bass.AP(tensor=rfs_ap.tensor, offset=rfs_ap.offset,
                                ap=list(rfs_ap.ap) + [[0, HD]])
                eq_v = eq[:].rearrange("p (h e) -> p h e", e=HD)
                nc.vector.tensor_tensor(eq_v, eq_v, rfs_b, op=mybir.AluOpType.mult)
                # qT via PE transposes into a 1-bank bf16 PSUM tile, then one
                # strided DVE copy out to qtall.
                tp = psum.tile([128, NPAIR, 128], BF16, tag="tp", name="tp", bufs=2)
                for p in range(NPAIR):
                    nc.tensor.transpose(tp[:, p, :], eq[:, p * 128:(p + 1) * 128],
                                        ident_sb)
                nc.vector.tensor_copy(qtall[:, :, tsl], tp)

            # ---- mask kv (d-major), transpose pairs on the PE, fold with
            # w_proj into M; 1/Z applied as M's per-partition row scale ----
            rzs = accp.tile([128, KC], F32, tag="rzs")
            nc.vector.reciprocal(rzs, zcols)
            kvsb = accp.tile([128, D], BF16, tag="kvsb")
            nc.gpsimd.tensor_mul(kvsb, kvred, mask_sb)
            ktp = psum.tile([128, NPAIR, 128], BF16, tag="ktp", name="ktp", bufs=1)
            for p in range(NPAIR):
                nc.tensor.transpose(ktp[:, p, :], kvsb[:, p * 128:(p + 1) * 128],
                                    ident_sb)
            kvt = accp.tile([128, D], BF16, tag="kvt")
            nc.vector.tensor_copy(kvt[:].rearrange("p (g e) -> p g e", e=128), ktp)
            m_sb = mpool.tile([128, KC, D], BF16, tag="m_sb")
            for p in range(NPAIR):
                for half in range(2):
                    sl = slice(half * 512, (half + 1) * 512)
                    mps = psum.tile([128, 512], F32, tag="mk", name="mps", bufs=1)
                    nc.tensor.matmul(mps, kvt[:, p * 128:(p + 1) * 128],
                                     wp_sb[p][:, sl], start=True, stop=True)
                    # M_p rows are d-local of pair p: scale by 1/Z[128p + r]
                    nc.scalar.mul(m_sb[:, p, sl], mps, rzs[:, p:p + 1])
            psum.release()

            # ---- Phase 4: out projection y = qT.T @ M ----
            psum = tc.alloc_tile_pool(name="psum4", bufs=1, space="PSUM")
            for tb in range(TB):
                tsl = slice(tb * 128, (tb + 1) * 128)
                yps = psum.tile([128, D], F32, tag="proj", name="yps", bufs=2)
                for half in range(2):
                    sl = slice(half * 512, (half + 1) * 512)
                    for c in range(KC):
                        nc.tensor.matmul(yps[:, sl], qtall[:, c, tsl],
                                         m_sb[:, c, sl],
                                         start=(c == 0), stop=(c == KC - 1))
                yt = ytp.tile([128, D], F32, tag="yt")
                if with_bias:
                    nc.vector.tensor_add(yt, yps, by_sb)
                else:
                    nc.scalar.copy(yt, yps)
                nc.gpsimd.dma_start(y[tsl, :], yt)
            psum.release()

    nc.compile()
    return nc


_NC = {}


def _get_nc(with_bias=False):
    if with_bias not in _NC:
        _NC[with_bias] = build_program(with_bias=with_bias)
    return _NC[with_bias]


def kernel(x, w_qkv, b_qkv, w_proj, b_proj):
    x = np.asarray(x, dtype=np.float32)
    w_qkv = np.asarray(w_qkv, dtype=np.float32)
    b_qkv = np.asarray(b_qkv, dtype=np.float32)
    w_proj = np.asarray(w_proj, dtype=np.float32)
    b_proj = np.asarray(b_proj, dtype=np.float32)

    bs, seqlen, dim = x.shape
    half = seqlen // 2
    bf = ml_dtypes.bfloat16

    wqm = np.ascontiguousarray(w_qkv[0:D].T.astype(bf))
    wkm = np.ascontiguousarray(w_qkv[D:2 * D].T.astype(bf))
    wvm = np.ascontiguousarray(w_qkv[2 * D:3 * D].T.astype(bf))
    wpm = np.ascontiguousarray(w_proj.T.astype(bf))
    bq, bv = b_qkv[0:D], b_qkv[2 * D:3 * D]

    ident = np.eye(128, dtype=bf)
    # mask[e_local, d_local] per 128-col pair block: head-diagonal blocks
    blk = np.zeros((128, 128), np.float32)
    blk[0:64, 0:64] = 1.0
    blk[64:128, 64:128] = 1.0
    maskc = np.ascontiguousarray(np.tile(blk, (1, KC)).astype(bf))

    with_bias = bool(np.any(b_qkv)) or bool(np.any(b_proj))

    in_maps = []
    for c in range(N_CORES):
        b, s = divmod(c, 2)
        chunk = np.ascontiguousarray(x[b, s * half:(s + 1) * half, :].T.astype(bf))
        im = {"xt": chunk, "wq": wqm, "wk": wkm, "wv": wvm, "wp": wpm,
              "maskd": maskc, "identd": ident}
        if with_bias:
            im["ebq"] = np.exp(bq).astype(np.float32)
            im["by"] = (SCALE * (w_proj @ bv) + b_proj).astype(np.float32)
        in_maps.append(im)

    nc = _get_nc(with_bias)
    global _last_in_maps
    _last_in_maps = in_maps
    res = bass_utils.run_bass_kernel_spmd(nc, in_maps, core_ids=list(range(N_CORES)))

    out = np.empty((bs, seqlen, dim), dtype=np.float32)
    for c in range(N_CORES):
        b, s = divmod(c, 2)
        out[b, s * half:(s + 1) * half, :] = res.results[c]["y"]
    return out
